# revision 34
# baseline (speedup 1.0000x reference)
"""Self dot-product attention kernel for Trainium2 (Bass/Tile), 8-core data parallel.

Problem: seq [32, 2048, 128] f32 ->
  attn = softmax(seq @ seq^T, axis=2); out = attn @ seq    (per batch)

Sharding: batch dim 32 -> 8 cores x 4 batches. No cross-core communication.

Per-core algorithm (per batch b, L=2048, C=128, NJ=16 row-tiles of 128):
  Xn [128p, NJ, 129] bf16: natural-layout X (cast in flight by SWDGE DMA)
     with a ones column at c=128.
  XT: X^T in bf16 as 4 chunk tiles [128p(c), 512] built with 16 per-tile
     TensorE transposes (chunked so early matmuls need not wait on the
     whole transpose wave).
  Phase 1 (per row-tile j): S^T_j = (XT[:, j]).T @ XT -> PSUM f32, in 2 chunks
     of [128, 1024]; E_j = exp(S^T_j - SHIFT) -> SBUF bf16 (one ACT instr per
     chunk).  S is symmetric and SHIFT global, so E rows here are E columns.
  Phase 2 (per row-tile i): O_i = sum_j E_j[:, l_i].T @ Xn[:, j, :] (PSUM f32).
     The ones column makes O_i[:, 128] = sum_m E[l_i, m] = softmax denominator.
     out[l_i, :] = O_i[:, :128] * (1 / O_i[:, 128])  (DVE recip + scalar mul).
  The softmax max-subtraction cancels in the division; the global SHIFT only
  keeps exp() in fp32/bf16 range (valid iff max(S)-SHIFT <= ~85 and
  min_l max_m S[l,m] - SHIFT >= ~-85; S diag dominates, row sumsq in
  [73.9, 203.1] for this input).  Phases of consecutive batches interleave so
  PE/ACT/DVE/DMA overlap.
"""

import numpy as np

B, L, C = 32, 2048, 128
NCORES = 8
BPC = B // NCORES  # batches per core
NJ = L // 128  # row tiles per batch
DEFAULT_SHIFT = 140.0

_CACHE = {}


def _build_bass(shift: float):
    import concourse.bacc as bacc
    import concourse.mybir as mybir
    import concourse.tile as tile
    from concourse.masks import make_identity

    dt = mybir.dt
    AF = mybir.ActivationFunctionType

    nc = bacc.Bacc(None, target_bir_lowering=False)
    x = nc.dram_tensor("x", [BPC, L, C], dt.float32, kind="ExternalInput")
    out = nc.dram_tensor("out", [BPC, L, C], dt.float32, kind="ExternalOutput")

    with tile.TileContext(nc) as tc:
        with (
            tc.tile_pool(name="xt", bufs=2 * 4) as xt_pool,
            tc.tile_pool(name="xn", bufs=12) as xn_pool,
            tc.tile_pool(name="xs", bufs=8) as xs_pool,
            tc.tile_pool(name="pt", bufs=2 * NJ) as pt_pool,
            tc.tile_pool(name="tmp", bufs=8) as tmp_pool,
            tc.tile_pool(name="osb", bufs=8) as osb_pool,
            tc.tile_pool(name="pa", bufs=16) as pa_pool,
            tc.tile_pool(name="ident", bufs=1) as ident_pool,
            tc.tile_pool(name="s_ps", bufs=2, space="PSUM") as s_pool,
            tc.tile_pool(name="ot_ps", bufs=4, space="PSUM") as ot_pool,
        ):
            ident = ident_pool.tile([128, 128], dt.bfloat16)

            NCH = 4  # Xn DMA chunks per batch
            JC = NJ // NCH  # j-tiles per chunk

            def stage_dma(b):
                """Start batch b's input DMAs; f32 lands in staging and DVE
                casts to bf16 (HWDGE issue is ~10x faster than SWDGE, and the
                cast-in-DMA path would force slow SWDGE issue).

                Xn comes in NCH per-chunk tiles so the first transposes can
                start before the whole megabyte lands (prologue latency)."""
                Xn = []
                xr = x[b].rearrange("(j p) c -> p j c", p=128)
                for q in range(NCH):
                    Xs = xs_pool.tile([128, JC, C], dt.float32, tag="xs")
                    nc.sync.dma_start(out=Xs, in_=xr[:, q * JC:(q + 1) * JC, :])
                    Xq = xn_pool.tile([128, JC, C + 2], dt.bfloat16, tag="xn")
                    nc.vector.tensor_copy(out=Xq[:, :, 0:C], in_=Xs)
                    nc.vector.memset(Xq[:, :, C:C + 2], 1.0)
                    Xn.append(Xq)
                XT = [
                    xt_pool.tile([128, 512], dt.bfloat16, tag="xt", name=f"XT{b}_{q}")
                    for q in range(NCH)
                ]
                return XT, Xn

            def emit_transpose(XT, Xn, j):
                """XT chunk col j = X[j-tile].T via TensorE + DVE copy."""
                tp = ot_pool.tile([128, 128], dt.bfloat16, tag="ot")
                nc.tensor.transpose(tp, Xn[j // JC][:, j % JC, 0:C], ident)
                q, jj = j // JC, j % JC
                nc.vector.tensor_copy(out=XT[q][:, jj * 128:(jj + 1) * 128], in_=tp)

            def phase1_chunk(XT, PT, j, c2):
                """One [128,1024] chunk of E^T row-tile j."""
                S = s_pool.tile([128, 1024], dt.float32, tag="s")
                lq, lj = j // JC, j % JC
                # One PSUM bank per matmul output: N=512.
                for q in range(2):
                    nc.tensor.matmul(
                        S[:, q * 512:(q + 1) * 512],
                        lhsT=XT[lq][:, lj * 128:(lj + 1) * 128],
                        rhs=XT[c2 * 2 + q],
                        start=True,
                        stop=True,
                    )
                nc.scalar.activation(
                    out=PT[:, c2 * 1024:(c2 + 1) * 1024],
                    in_=S[:, :],
                    func=AF.Exp,
                    bias=-shift,
                    scale=1.0,
                )

            def phase1_j(XT, j, PTs):
                """Row-tile j of E^T = exp(S^T - shift) -> bf16 SBUF."""
                PT = pt_pool.tile([128, L], dt.bfloat16, tag="pt")
                for c2 in range(2):
                    phase1_chunk(XT, PT, j, c2)
                PTs.append(PT)

            def phase2_i(b, Xn, i, PTs):
                """Output row-tile i of batch b: O_i = P_i @ [X | 1], normalized."""
                O = ot_pool.tile([128, 132], dt.float32, tag="ot")
                for j in range(NJ):
                    nc.tensor.matmul(
                        O[:, 0:C + 2],
                        lhsT=PTs[j][:, i * 128:(i + 1) * 128],
                        rhs=Xn[j // JC][:, j % JC, :],
                        start=(j == 0),
                        stop=(j == NJ - 1),
                    )
                rinv = tmp_pool.tile([128, 1], dt.float32, tag="rinv")
                nc.vector.reciprocal(rinv, O[:, C:C + 1])
                osb = osb_pool.tile([128, C], dt.float32, tag="osb")
                nc.vector.tensor_scalar_mul(osb, O[:, 0:C], rinv)
                nc.sync.dma_start(out=out[b, i * 128:(i + 1) * 128, :], in_=osb)

            def phase2_last(b, Xn, PTs):
                """Tail-batch phase 2, split so only one matmul per output
                tile depends on the final exp: j=0..14 accumulate and drain
                to SBUF while phase 1 is still running; j=15 lands after."""
                partials = []
                for i in range(NJ):
                    OA = ot_pool.tile([128, 132], dt.float32, tag="ot")
                    for j in range(NJ - 1):
                        nc.tensor.matmul(
                            OA[:, 0:C + 1],
                            lhsT=PTs[j][:, i * 128:(i + 1) * 128],
                            rhs=Xn[j // JC][:, j % JC, :],
                            start=(j == 0),
                            stop=(j == NJ - 2),
                        )
                    pa = pa_pool.tile([128, C + 1], dt.float32, tag="pa")
                    nc.vector.tensor_copy(out=pa, in_=OA[:, 0:C + 1])
                    partials.append(pa)
                for i in range(NJ):
                    OB = ot_pool.tile([128, 132], dt.float32, tag="ot")
                    j = NJ - 1
                    nc.tensor.matmul(
                        OB[:, 0:C + 1],
                        lhsT=PTs[j][:, i * 128:(i + 1) * 128],
                        rhs=Xn[j // JC][:, j % JC, :],
                        start=True,
                        stop=True,
                    )
                    osum = osb_pool.tile([128, C + 1], dt.float32, tag="osum")
                    nc.vector.tensor_add(osum, partials[i], OB[:, 0:C + 1])
                    rinv = tmp_pool.tile([128, 1], dt.float32, tag="rinv")
                    nc.vector.reciprocal(rinv, osum[:, C:C + 1])
                    osb = osb_pool.tile([128, C], dt.float32, tag="osb")
                    nc.vector.tensor_scalar_mul(osb, osum[:, 0:C], rinv)
                    nc.sync.dma_start(out=out[b, i * 128:(i + 1) * 128, :], in_=osb)

            # Software pipeline across batches: phase2(b-1) interleaved with
            # phase1(b) so PE fills ACT-wait gaps and vice versa.  The next
            # batch's PE transposes are emitted late in the current batch so
            # the PE never queues behind an in-flight input DMA.
            XT, Xn = stage_dma(0)
            make_identity(nc, ident)
            # Batch-0 prologue: first row-tile's chunks interleave with the
            # transpose wave so the PE starts matmuls as soon as the first
            # half of XT exists.
            for j in range(NJ // 2):
                emit_transpose(XT, Xn, j)
            PT0 = pt_pool.tile([128, L], dt.bfloat16, tag="pt")
            phase1_chunk(XT, PT0, 0, 0)
            for j in range(NJ // 2, NJ):
                emit_transpose(XT, Xn, j)
            phase1_chunk(XT, PT0, 0, 1)
            prev = None  # (b, Xn, PTs) of the previous batch
            for b in range(BPC):
                PTs = [PT0] if b == 0 else []
                if b + 1 < BPC:
                    nxt = stage_dma(b + 1)
                for k in range(NJ):
                    if b == 0 and k == 0:
                        continue  # emitted in the prologue above
                    phase1_j(XT, k, PTs)
                    if prev is not None:
                        phase2_i(prev[0], prev[1], k, prev[2])
                    if b + 1 < BPC and k >= NJ // 2:
                        emit_transpose(nxt[0], nxt[1], 2 * (k - NJ // 2))
                        emit_transpose(nxt[0], nxt[1], 2 * (k - NJ // 2) + 1)
                prev = (b, Xn, PTs)
                if b + 1 < BPC:
                    XT, Xn = nxt
            for k in range(NJ):
                phase2_i(prev[0], prev[1], k, prev[2])

    nc.compile()
    return nc


def _get_nc(shift: float):
    if shift not in _CACHE:
        _CACHE[shift] = _build_bass(shift)
    return _CACHE[shift]


def kernel(seq: np.ndarray) -> np.ndarray:
    from concourse.bass_utils import run_bass_kernel_spmd

    seq = np.ascontiguousarray(np.asarray(seq, dtype=np.float32))
    assert seq.shape == (B, L, C), seq.shape

    # Pick the exp shift from the data (midpoint of the valid window); baked
    # into the NEFF as an immediate, so quantize coarsely to keep cache hits.
    sumsq = np.einsum("blc,blc->bl", seq, seq)
    lo, hi = float(sumsq.max()) - 80.0, float(sumsq.min()) + 80.0
    shift = round(float(np.clip(DEFAULT_SHIFT, lo, hi)))

    nc = _get_nc(shift)
    in_maps = [{"x": seq[k * BPC:(k + 1) * BPC]} for k in range(NCORES)]
    res = run_bass_kernel_spmd(nc, in_maps, core_ids=list(range(NCORES)))
    return np.concatenate([r["out"] for r in res.results], axis=0)
